# revision 1
# baseline (speedup 1.0000x reference)
"""Cross-view attention (nn_CrossViewAttention) Trainium2 Bass kernel.

Reference computation (B=2, N=4096, D=512):
    co    = relu(concat([x_f, x_s], -1) @ Wc.T + bc)
    out_f = attend(x_f@Wq.T+bq, x_s@Wk.T+bk, x_f@Wv.T+bv) + co
    out_s = attend(x_s@Wq.T+bq, x_f@Wk.T+bk, x_s@Wv.T+bv) + co
    attend(Q,K,V) = (softmax(Q K^T) / L1 / sqrt(D)) @ V

Sharding: 8 cores = (direction f/s) x (batch 0/1) x (sequence half).
Each core computes 2048 output rows of one direction against the full
4096-row K/V for its (direction, batch), SPMD with per-core input data.
Rows are permuted host-side so every core's own rows come first; the
attention reduction over keys is permutation invariant.

On-core schedule (all matmuls bf16, fp32 PSUM accumulation):
  phase 1: PE-transpose x_A/x_B into feature-major bf16 copies, project
           Q^T/K^T (bias via per-partition activation add), V, and the
           co-occurrence MLP (bias via rank-1 ones matmul, relu on ACT).
  phase 2: per 128-row query block: scores = Q^T-block.T @ K^T chunks,
           exp(s - 40) on ACT with accumulated row sums (softmax is
           shift invariant; scores here are empirically < 40 so no
           row-max pass is needed), PE-transpose the probabilities,
           PV matmul, then scale by 1/(rowsum*sqrt(D)) and add co.
"""

import sys
from contextlib import ExitStack

for _p in ("/opt/trn_rl_repo", "/root/.axon_site/_ro/trn_rl_repo"):
    if _p not in sys.path:
        sys.path.insert(0, _p)

import ml_dtypes
import numpy as np

import concourse.bacc as bacc
import concourse.bass as bass
import concourse.mybir as mybir
import concourse.tile as tile
from concourse.masks import make_identity

P = 128
D = 512
DC = D // P  # contraction chunks of 128
SQRT_D = float(np.sqrt(D))
EXP_SHIFT = -40.0

F32 = mybir.dt.float32
BF16 = mybir.dt.bfloat16
AF = mybir.ActivationFunctionType


def build_program(nq, nkv, reps=1, pair_split=None):
    """pair_split: each core of an (direction, batch) pair projects K/V and
    transposes x only for its own half; an AllGather over core pairs
    [[0,1],[2,3],[4,5],[6,7]] shares the halves. Requires nkv == 2*nq and the
    8-core in_map layout from make_in_maps (half h is group rank h)."""
    if pair_split is None:
        pair_split = nkv == 2 * nq
    nc = bacc.Bacc("TRN2", target_bir_lowering=False, debug=False, num_devices=8)

    xA = nc.dram_tensor("xA", [nkv, D], F32, kind="ExternalInput").ap()
    xB = nc.dram_tensor("xB", [nkv, D], F32, kind="ExternalInput").ap()
    wqT = nc.dram_tensor("wqT", [D, D], BF16, kind="ExternalInput").ap()
    wkT = nc.dram_tensor("wkT", [D, D], BF16, kind="ExternalInput").ap()
    wvT = nc.dram_tensor("wvT", [D, D], BF16, kind="ExternalInput").ap()
    wcAT = nc.dram_tensor("wcAT", [D, D], BF16, kind="ExternalInput").ap()
    wcBT = nc.dram_tensor("wcBT", [D, D], BF16, kind="ExternalInput").ap()
    bq = nc.dram_tensor("bq", [D], F32, kind="ExternalInput").ap()
    bk = nc.dram_tensor("bk", [D], F32, kind="ExternalInput").ap()
    bv = nc.dram_tensor("bv", [D], F32, kind="ExternalInput").ap()
    bc = nc.dram_tensor("bc", [D], F32, kind="ExternalInput").ap()
    out = nc.dram_tensor("out", [nq, D], F32, kind="ExternalOutput").ap()
    co_dram = nc.dram_tensor("co_scratch", [nq, D], F32).ap()
    sums_dram = nc.dram_tensor("sums_scratch", [nq], F32).ap()

    NBQ = nq // P  # query row blocks
    MCK = nkv // P  # key row chunks
    MB = nkv // 512  # score column blocks

    with tile.TileContext(nc) as tc:
        for _rep in range(reps):
            _emit_body(
                nc, tc, xA, xB, wqT, wkT, wvT, wcAT, wcBT, bq, bk, bv, bc,
                out, co_dram, sums_dram, nq, nkv, NBQ, MCK, MB, pair_split, _rep,
            )

    nc.compile()
    return nc


def _emit_body(
    nc, tc, xA, xB, wqT, wkT, wvT, wcAT, wcBT, bq, bk, bv, bc,
    out, co_dram, sums_dram, nq, nkv, NBQ, MCK, MB, pair_split, rep,
):
    nhalf = nq if pair_split else nkv  # rows of x transposed / K,V projected
    if pair_split:
        KV_K = DC * nhalf  # kT-half bf16 elements per partition
        KV_V = (nhalf // P) * D
        k_mine = nc.dram_tensor(f"k_mine_{rep}", [P, KV_K], BF16).ap()
        k_all = nc.dram_tensor(f"k_all_{rep}", [2, P, KV_K], BF16).ap()
        v_mine = nc.dram_tensor(f"v_mine_{rep}", [P, KV_V], BF16).ap()
        v_all = nc.dram_tensor(f"v_all_{rep}", [2, P, KV_V], BF16).ap()
    with ExitStack() as st:
        persist = st.enter_context(tc.tile_pool(name="persist", bufs=1))

        ident_f = persist.tile([P, P], F32, name="ident_f")
        make_identity(nc, ident_f)

        w_sb = {}
        for nm, ap_ in (
            ("wq", wqT),
            ("wk", wkT),
            ("wv", wvT),
            ("wcA", wcAT),
            ("wcB", wcBT),
        ):
            t = persist.tile([P, DC, D], BF16, name=f"w_{nm}")
            nc.sync.dma_start(out=t, in_=ap_.rearrange("(c p) o -> p c o", p=P))
            w_sb[nm] = t

        bq_sb = persist.tile([P, DC], F32, name="bq_sb")
        bk_sb = persist.tile([P, DC], F32, name="bk_sb")
        for ob in range(DC):
            nc.sync.dma_start(
                out=bq_sb[:, ob : ob + 1], in_=bq[ob * P : (ob + 1) * P][:, None]
            )
            nc.sync.dma_start(
                out=bk_sb[:, ob : ob + 1], in_=bk[ob * P : (ob + 1) * P][:, None]
            )

        bv_bc = persist.tile([P, D], F32, name="bv_bc")
        nc.sync.dma_start(
            out=bv_bc,
            in_=bass.AP(tensor=bv.tensor, offset=bv.offset, ap=[[0, P]] + list(bv.ap)),
        )
        bc_bc = persist.tile([P, D], F32, name="bc_bc")
        nc.sync.dma_start(
            out=bc_bc,
            in_=bass.AP(tensor=bc.tensor, offset=bc.offset, ap=[[0, P]] + list(bc.ap)),
        )
        ones_col = persist.tile([P, 1], BF16, name="ones_col")
        nc.vector.memset(ones_col, 1.0)
        shift_sb = persist.tile([P, 1], F32, name="shift_sb")
        nc.vector.memset(shift_sb, EXP_SHIFT)

        qT_sb = persist.tile([P, DC, nq], BF16, name="qT_sb")
        kT_sb = persist.tile([P, DC, nkv], BF16, name="kT_sb")
        v_sb = persist.tile([P, MCK, D], BF16, name="v_sb")

        # ---------------- phase 1: transposes + projections ----------------
        with ExitStack() as ph1:
            xt_pool = ph1.enter_context(tc.tile_pool(name="xt", bufs=1))
            xn_pool = ph1.enter_context(tc.tile_pool(name="xn", bufs=4))
            co_pool = ph1.enter_context(tc.tile_pool(name="cop", bufs=3))
            ps1 = ph1.enter_context(tc.tile_pool(name="ps1", bufs=4, space="PSUM"))
            tp1 = ph1.enter_context(tc.tile_pool(name="tp1", bufs=2, space="PSUM"))

            xAT = xt_pool.tile([P, DC, nhalf], BF16, name="xAT")
            xBT = xt_pool.tile([P, DC, nhalf], BF16, name="xBT")

            for src_ap, dstT in ((xA, xAT), (xB, xBT)):
                for nt in range(nhalf // P):
                    xn = xn_pool.tile([P, D], F32, name="xn", tag="xn")
                    nc.sync.dma_start(out=xn, in_=src_ap[nt * P : (nt + 1) * P, :])
                    tp = tp1.tile([P, DC, P], F32, name="tp", tag="tp")
                    for c in range(DC):
                        nc.tensor.transpose(
                            tp[:, c, :], xn[:, c * P : (c + 1) * P], ident_f
                        )
                    nc.vector.tensor_copy(
                        out=dstT[:, :, nt * P : (nt + 1) * P], in_=tp
                    )

            # K first so the pair AllGather launches as early as possible
            if pair_split:
                kvK_stage = xt_pool.tile([P, DC, nhalf], BF16, name="kvK_stage")
                kvV_stage = xt_pool.tile([P, nhalf // P, D], BF16, name="kvV_stage")
            for ob in range(DC):
                for s0 in range(0, nhalf, 512):
                    w = min(512, nhalf - s0)
                    ps = ps1.tile([P, 512], F32, name="ps_k", tag="ps1")
                    for c in range(DC):
                        nc.tensor.matmul(
                            ps[:, :w],
                            lhsT=w_sb["wk"][:, c, ob * P : (ob + 1) * P],
                            rhs=xBT[:, c, s0 : s0 + w],
                            start=(c == 0),
                            stop=(c == DC - 1),
                        )
                    kdst = kvK_stage if pair_split else kT_sb
                    nc.scalar.activation(
                        out=kdst[:, ob, s0 : s0 + w],
                        in_=ps[:, :w],
                        func=AF.Identity,
                        bias=bk_sb[:, ob : ob + 1],
                        scale=1.0,
                    )

            # V in natural [m, o] layout; bv is deferred to the output tiles
            for m in range(nhalf // P):
                ps = ps1.tile([P, 512], F32, name="ps_v", tag="ps1")
                for c in range(DC):
                    nc.tensor.matmul(
                        ps,
                        lhsT=xAT[:, c, m * P : (m + 1) * P],
                        rhs=w_sb["wv"][:, c, :],
                        start=(c == 0),
                        stop=(c == DC - 1),
                    )
                vdst = kvV_stage if pair_split else v_sb
                nc.scalar.activation(out=vdst[:, m, :], in_=ps, func=AF.Copy)

            if pair_split:
                MH = nhalf // P
                # K gather first: scores only need kT_sb, so phase 2 can start
                # while the V gather is still in flight.
                nc.sync.dma_start(out=k_mine, in_=kvK_stage)
                nc.gpsimd.collective_compute(
                    "AllGather",
                    mybir.AluOpType.bypass,
                    replica_groups=[[0, 1], [2, 3], [4, 5], [6, 7]],
                    ins=[k_mine],
                    outs=[k_all],
                )
                for h in range(2):
                    nc.sync.dma_start(
                        out=kT_sb[:, :, h * nhalf : (h + 1) * nhalf],
                        in_=k_all[h].rearrange("p (c m) -> p c m", c=DC),
                    )
                nc.sync.dma_start(out=v_mine, in_=kvV_stage)
                nc.gpsimd.collective_compute(
                    "AllGather",
                    mybir.AluOpType.bypass,
                    replica_groups=[[0, 1], [2, 3], [4, 5], [6, 7]],
                    ins=[v_mine],
                    outs=[v_all],
                )
                for h in range(2):
                    nc.sync.dma_start(
                        out=v_sb[:, h * MH : (h + 1) * MH, :],
                        in_=v_all[h].rearrange("p (m o) -> p m o", m=MH),
                    )

            # Q^T (own rows), bias added on the ACT copy
            for ob in range(DC):
                for s0 in range(0, nq, 512):
                    w = min(512, nq - s0)
                    ps = ps1.tile([P, 512], F32, name="ps_q", tag="ps1")
                    for c in range(DC):
                        nc.tensor.matmul(
                            ps[:, :w],
                            lhsT=w_sb["wq"][:, c, ob * P : (ob + 1) * P],
                            rhs=xAT[:, c, s0 : s0 + w],
                            start=(c == 0),
                            stop=(c == DC - 1),
                        )
                    nc.scalar.activation(
                        out=qT_sb[:, ob, s0 : s0 + w],
                        in_=ps[:, :w],
                        func=AF.Identity,
                        bias=bq_sb[:, ob : ob + 1],
                        scale=1.0,
                    )

            # co = relu(xA@WcA.T + xB@WcB.T + bc) + bv -> DRAM scratch
            for nb in range(NBQ):
                ps = ps1.tile([P, 512], F32, name="ps_c", tag="ps1")
                for c in range(DC):
                    nc.tensor.matmul(
                        ps,
                        lhsT=xAT[:, c, nb * P : (nb + 1) * P],
                        rhs=w_sb["wcA"][:, c, :],
                        start=(c == 0),
                        stop=False,
                    )
                for c in range(DC):
                    nc.tensor.matmul(
                        ps,
                        lhsT=xBT[:, c, nb * P : (nb + 1) * P],
                        rhs=w_sb["wcB"][:, c, :],
                        start=False,
                        stop=(c == DC - 1),
                    )
                cadd = co_pool.tile([P, D], F32, name="cadd", tag="cadd")
                nc.vector.tensor_add(cadd, ps, bc_bc)
                cot = co_pool.tile([P, D], F32, name="cot", tag="cot")
                nc.scalar.activation(out=cot, in_=cadd, func=AF.Relu)
                nc.vector.tensor_add(cot, cot, bv_bc)
                nc.sync.dma_start(out=co_dram[nb * P : (nb + 1) * P, :], in_=cot)

        # ---------------- phase 2: attention (S^T layout) ----------------
        # Scores are computed transposed (keys on partitions): the exp output
        # is already the [key, query] layout the PV matmul needs as its
        # stationary operand, so no PE transposes of the attention matrix.
        # Row sums come from a ones-stationary matmul over the same tiles,
        # bounced through DRAM to become per-partition scale factors.
        at_pool = st.enter_context(tc.tile_pool(name="at_pool", bufs=2))
        o_pool = st.enter_context(tc.tile_pool(name="o_pool", bufs=3))
        r_pool = st.enter_context(tc.tile_pool(name="r_pool", bufs=3))
        sps_pool = st.enter_context(tc.tile_pool(name="sps", bufs=3, space="PSUM"))
        sum_pool = st.enter_context(tc.tile_pool(name="sump", bufs=2, space="PSUM"))
        pv_pool = st.enter_context(tc.tile_pool(name="pv", bufs=2, space="PSUM"))

        for s0 in range(0, nq, 512):
            w = min(512, nq - s0)
            at_sb = at_pool.tile([P, MCK, 512], BF16, name="at_sb", tag="at")
            for mb in range(MCK):
                sps = sps_pool.tile([P, 512], F32, name="sps", tag="sps")
                for c in range(DC):
                    nc.tensor.matmul(
                        sps[:, :w],
                        lhsT=kT_sb[:, c, mb * P : (mb + 1) * P],
                        rhs=qT_sb[:, c, s0 : s0 + w],
                        start=(c == 0),
                        stop=(c == DC - 1),
                    )
                nc.scalar.activation(
                    out=at_sb[:, mb, :w],
                    in_=sps[:, :w],
                    func=AF.Exp,
                    bias=shift_sb,
                    scale=1.0,
                )
            # L1 sums over keys: ones^T @ A^T, accumulated across key chunks
            ssum = sum_pool.tile([1, 512], F32, name="ssum", tag="ssum")
            for mb in range(MCK):
                nc.tensor.matmul(
                    ssum[:, :w],
                    lhsT=ones_col,
                    rhs=at_sb[:, mb, :w],
                    start=(mb == 0),
                    stop=(mb == MCK - 1),
                )
            sums_row = r_pool.tile([1, 512], F32, name="sums_row", tag="smr")
            nc.scalar.activation(out=sums_row[:, :w], in_=ssum[:, :w], func=AF.Copy)
            nc.sync.dma_start(out=sums_dram[s0 : s0 + w], in_=sums_row[:, :w])
            nj = w // P
            sums_col = r_pool.tile([P, 4], F32, name="sums_col", tag="smc")
            nc.sync.dma_start(
                out=sums_col[:, :nj],
                in_=sums_dram[s0 : s0 + w].rearrange("(j p) -> p j", p=P),
            )
            nc.scalar.mul(sums_col[:, :nj], sums_col[:, :nj], SQRT_D)
            rs_col = r_pool.tile([P, 4], F32, name="rs_col", tag="rsc")
            nc.vector.reciprocal(out=rs_col[:, :nj], in_=sums_col[:, :nj])

            for j in range(nj):
                pv = pv_pool.tile([P, D], F32, name="pv", tag="pv")
                for mb in range(MCK):
                    nc.tensor.matmul(
                        pv,
                        lhsT=at_sb[:, mb, j * P : (j + 1) * P],
                        rhs=v_sb[:, mb, :],
                        start=(mb == 0),
                        stop=(mb == MCK - 1),
                    )
                attn = o_pool.tile([P, D], F32, name="attn", tag="attn")
                nc.scalar.activation(
                    out=attn, in_=pv, func=AF.Copy, scale=rs_col[:, j : j + 1]
                )
                r0 = s0 + j * P
                cot2 = o_pool.tile([P, D], F32, name="cot2", tag="cot2")
                nc.sync.dma_start(out=cot2, in_=co_dram[r0 : r0 + P, :])
                outt = o_pool.tile([P, D], F32, name="outt", tag="outt")
                nc.vector.tensor_add(outt, attn, cot2)
                nc.sync.dma_start(out=out[r0 : r0 + P, :], in_=outt)


_PROG_CACHE = {}


def _get_program(nq, nkv):
    key = (nq, nkv)
    if key not in _PROG_CACHE:
        _PROG_CACHE[key] = build_program(nq, nkv)
    return _PROG_CACHE[key]


def make_in_maps(x_f, x_s, Wq, bq, Wk, bk, Wv, bv, Wc, bc):
    """Per-core SPMD input dicts + (direction, batch, half) layout."""
    x_f = np.asarray(x_f, np.float32)
    x_s = np.asarray(x_s, np.float32)
    B, N, _ = x_f.shape
    nq = N // 2
    bf = ml_dtypes.bfloat16
    WqT = np.ascontiguousarray(np.asarray(Wq, np.float32).T).astype(bf)
    WkT = np.ascontiguousarray(np.asarray(Wk, np.float32).T).astype(bf)
    WvT = np.ascontiguousarray(np.asarray(Wv, np.float32).T).astype(bf)
    Wc = np.asarray(Wc, np.float32)
    WcfT = np.ascontiguousarray(Wc[:, :D].T).astype(bf)
    WcsT = np.ascontiguousarray(Wc[:, D:].T).astype(bf)
    bq32, bk32, bv32, bc32 = (
        np.ascontiguousarray(np.asarray(b, np.float32)) for b in (bq, bk, bv, bc)
    )
    # Reference computes attend(Q, K, x@Wv.T + bv) with attention rows summing
    # to 1/sqrt(D) after its post-softmax scaling, so bv contributes bv/sqrt(D).
    # The kernel adds its "bv" input to output rows directly — pre-scale here.
    bv32 = np.ascontiguousarray(bv32 / np.sqrt(D, dtype=np.float32))
    in_maps, layout = [], []
    for d in range(2):
        for b in range(B):
            for h in range(2):
                xq = x_f[b] if d == 0 else x_s[b]
                xk = x_s[b] if d == 0 else x_f[b]
                if h == 1:
                    idx = np.r_[nq:N, 0:nq]
                    xq, xk = xq[idx], xk[idx]
                in_maps.append(
                    {
                        "xA": np.ascontiguousarray(xq),
                        "xB": np.ascontiguousarray(xk),
                        "wqT": WqT,
                        "wkT": WkT,
                        "wvT": WvT,
                        "wcAT": WcfT if d == 0 else WcsT,
                        "wcBT": WcsT if d == 0 else WcfT,
                        "bq": bq32,
                        "bk": bk32,
                        "bv": bv32,
                        "bc": bc32,
                    }
                )
                layout.append((d, b, h))
    return in_maps, layout


def kernel(x_f, x_s, Wq, bq, Wk, bk, Wv, bv, Wc, bc):
    x_f = np.asarray(x_f, np.float32)
    B, N, _ = x_f.shape
    nq = N // 2
    nc = _get_program(nq, N)
    in_maps, layout = make_in_maps(x_f, x_s, Wq, bq, Wk, bk, Wv, bv, Wc, bc)

    from concourse.bass_utils import run_bass_kernel_spmd

    res = run_bass_kernel_spmd(nc, in_maps, list(range(len(in_maps))))
    out_f = np.empty((B, N, D), np.float32)
    out_s = np.empty((B, N, D), np.float32)
    for (d, b, h), r in zip(layout, res.results):
        tgt = out_f if d == 0 else out_s
        tgt[b, h * nq : (h + 1) * nq] = r["out"]
    return out_f, out_s



# revision 11
# speedup vs baseline: 1.1434x; 1.1434x over previous
"""Cross-view attention (nn_CrossViewAttention) Trainium2 Bass kernel.

Reference computation (B=2, N=4096, D=512):
    co    = relu(concat([x_f, x_s], -1) @ Wc.T + bc)
    out_f = attend(x_f@Wq.T+bq, x_s@Wk.T+bk, x_f@Wv.T+bv) + co
    out_s = attend(x_s@Wq.T+bq, x_f@Wk.T+bk, x_s@Wv.T+bv) + co
    attend(Q,K,V) = (softmax(Q K^T) / L1 / sqrt(D)) @ V

Sharding: 8 cores = (direction f/s) x (batch 0/1) x (sequence half).
Each core computes 2048 output rows of one direction against the full
4096-row K/V for its (direction, batch), SPMD with per-core input data.
Rows are permuted host-side so every core's own rows come first.

Key optimizations over the bf16 baseline:
  - x arrives pre-transposed (feature-major) in bf16 from the host: no
    on-device PE transposes.
  - Q/K are projected in bf16 but stored fp8-e4m3; the N x N score
    matmul runs in fp8 DoubleRow mode (K=256 per instruction).
  - Probabilities are L1-NORMALIZED on device (exp in bf16, row sums
    via a ones matmul, reciprocal broadcast via a DRAM bounce, DVE
    rescale) and stored fp8-e5m2 in [0,1]; the PV matmul also runs in
    fp8 DoubleRow.  Normalizing before PV removes the post-PV
    per-row scaling entirely.
  - The co-occurrence MLP is split across direction pairs (it is
    identical for both directions) and shared via AllGather; K/V are
    split across sequence-half pairs as in the baseline.  co lives in
    SBUF (no DRAM bounce).
  - Output is one fused DVE op: out = pv * (1/sqrt(D)) + co.
"""

import sys
from contextlib import ExitStack

for _p in ("/opt/trn_rl_repo", "/root/.axon_site/_ro/trn_rl_repo"):
    if _p not in sys.path:
        sys.path.insert(0, _p)

import ml_dtypes
import numpy as np

import concourse.bacc as bacc
import concourse.bass as bass
import concourse.mybir as mybir
import concourse.tile as tile

P = 128
D = 512
DC = D // P  # contraction chunks of 128
SQRT_D = float(np.sqrt(D))
INV_SQRT_D = float(1.0 / np.sqrt(D))
EXP_SHIFT = -40.0

F32 = mybir.dt.float32
BF16 = mybir.dt.bfloat16
FP8_QK = mybir.dt.float8e4
FP8_AT = mybir.dt.float8e5
AF = mybir.ActivationFunctionType
DR = mybir.MatmulPerfMode.DoubleRow

KV_GROUPS = [[0, 1], [2, 3], [4, 5], [6, 7]]  # sequence-half pairs
CO_GROUPS = [[0, 4], [1, 5], [2, 6], [3, 7]]  # direction pairs


def build_program(nq, nkv, reps=1, fp8_scores=True):
    nc = bacc.Bacc("TRN2", target_bir_lowering=False, debug=False, num_devices=8)

    xAT = nc.dram_tensor("xAT", [D, nq], BF16, kind="ExternalInput").ap()
    xBT = nc.dram_tensor("xBT", [D, nq], BF16, kind="ExternalInput").ap()
    # this core's assigned co-occurrence rows (half of its query block;
    # direction pairs compute complementary halves and AllGather them)
    xcAT = nc.dram_tensor("xcAT", [D, nq // 2], BF16, kind="ExternalInput").ap()
    xcBT = nc.dram_tensor("xcBT", [D, nq // 2], BF16, kind="ExternalInput").ap()
    wqT = nc.dram_tensor("wqT", [D, D], BF16, kind="ExternalInput").ap()
    wkT = nc.dram_tensor("wkT", [D, D], BF16, kind="ExternalInput").ap()
    wvT = nc.dram_tensor("wvT", [D, D], BF16, kind="ExternalInput").ap()
    wcAT = nc.dram_tensor("wcAT", [D, D], BF16, kind="ExternalInput").ap()
    wcBT = nc.dram_tensor("wcBT", [D, D], BF16, kind="ExternalInput").ap()
    bq = nc.dram_tensor("bq", [D], F32, kind="ExternalInput").ap()
    bk = nc.dram_tensor("bk", [D], F32, kind="ExternalInput").ap()
    bv = nc.dram_tensor("bv", [D], F32, kind="ExternalInput").ap()
    bc = nc.dram_tensor("bc", [D], F32, kind="ExternalInput").ap()
    out = nc.dram_tensor("out", [nq, D], F32, kind="ExternalOutput").ap()

    with tile.TileContext(nc) as tc:
        for rep in range(reps):
            _emit_body(
                nc, tc, xAT, xBT, xcAT, xcBT, wqT, wkT, wvT, wcAT, wcBT,
                bq, bk, bv, bc, out, nq, nkv, fp8_scores, rep,
            )

    nc.compile()
    return nc


def _emit_body(
    nc, tc, xAT, xBT, xcAT, xcBT, wqT, wkT, wvT, wcAT, wcBT,
    bq, bk, bv, bc, out, nq, nkv, fp8_scores, rep,
):
    NBQ = nq // P   # query row blocks (16)
    MCK = nkv // P  # key row chunks (32)
    MH = nq // P    # per-half V row blocks (16)
    NCO = NBQ // 2  # co blocks computed locally (8)
    qk_dt = FP8_QK if fp8_scores else BF16

    # collective scratch (per rep)
    k_mine = nc.dram_tensor(f"k_mine_{rep}", [P, DC * nq], qk_dt).ap()
    k_all = nc.dram_tensor(f"k_all_{rep}", [2, P, DC * nq], qk_dt).ap()
    v_mine = nc.dram_tensor(f"v_mine_{rep}", [P, MH * D], FP8_AT).ap()
    v_all = nc.dram_tensor(f"v_all_{rep}", [2, P, MH * D], FP8_AT).ap()
    co_mine = nc.dram_tensor(f"co_mine_{rep}", [P, NCO * D], BF16).ap()
    co_all = nc.dram_tensor(f"co_all_{rep}", [2, P, NCO * D], BF16).ap()
    rs_dram = nc.dram_tensor(f"rs_dram_{rep}", [512], F32).ap()

    with ExitStack() as st:
        persist = st.enter_context(tc.tile_pool(name="persist", bufs=1))

        w_sb = {}
        for nm, ap_ in (
            ("wq", wqT), ("wk", wkT), ("wv", wvT), ("wcA", wcAT), ("wcB", wcBT),
        ):
            t = persist.tile([P, DC, D], BF16, name=f"w_{nm}")
            nc.sync.dma_start(out=t, in_=ap_.rearrange("(c p) o -> p c o", p=P))
            w_sb[nm] = t

        bq_sb = persist.tile([P, DC], F32, name="bq_sb")
        bk_sb = persist.tile([P, DC], F32, name="bk_sb")
        for ob in range(DC):
            nc.sync.dma_start(
                out=bq_sb[:, ob : ob + 1], in_=bq[ob * P : (ob + 1) * P][:, None]
            )
            nc.sync.dma_start(
                out=bk_sb[:, ob : ob + 1], in_=bk[ob * P : (ob + 1) * P][:, None]
            )

        bv_bc = persist.tile([P, D], F32, name="bv_bc")
        nc.sync.dma_start(
            out=bv_bc,
            in_=bass.AP(tensor=bv.tensor, offset=bv.offset, ap=[[0, P]] + list(bv.ap)),
        )
        bc_bc = persist.tile([P, D], F32, name="bc_bc")
        nc.sync.dma_start(
            out=bc_bc,
            in_=bass.AP(tensor=bc.tensor, offset=bc.offset, ap=[[0, P]] + list(bc.ap)),
        )
        ones_col = persist.tile([P, 1], BF16, name="ones_col")
        nc.vector.memset(ones_col, 1.0)
        shift_sb = persist.tile([P, 1], F32, name="shift_sb")
        nc.vector.memset(shift_sb, EXP_SHIFT)

        qT_sb = persist.tile([P, DC, nq], qk_dt, name="qT_sb")
        kT_sb = persist.tile([P, DC, nkv], qk_dt, name="kT_sb")
        v_sb = persist.tile([P, MCK, D], FP8_AT, name="v_sb")
        co_sb = persist.tile([P, NBQ, D], BF16, name="co_sb")

        # ---------------- phase 1: projections + gathers ----------------
        with ExitStack() as ph1:
            xst_pool = ph1.enter_context(tc.tile_pool(name="xst", bufs=1))
            st_pool = ph1.enter_context(tc.tile_pool(name="stg", bufs=1))
            co_pool = ph1.enter_context(tc.tile_pool(name="cop", bufs=3))
            ps1 = ph1.enter_context(tc.tile_pool(name="ps1", bufs=4, space="PSUM"))

            xAT_sb = xst_pool.tile([P, DC, nq], BF16, name="xAT_sb")
            nc.sync.dma_start(out=xAT_sb, in_=xAT.rearrange("(c p) n -> p c n", p=P))
            xBT_sb = xst_pool.tile([P, DC, nq], BF16, name="xBT_sb")
            nc.sync.dma_start(out=xBT_sb, in_=xBT.rearrange("(c p) n -> p c n", p=P))
            xcAT_sb = xst_pool.tile([P, DC, nq // 2], BF16, name="xcAT_sb")
            nc.sync.dma_start(out=xcAT_sb, in_=xcAT.rearrange("(c p) n -> p c n", p=P))
            xcBT_sb = xst_pool.tile([P, DC, nq // 2], BF16, name="xcBT_sb")
            nc.sync.dma_start(out=xcBT_sb, in_=xcBT.rearrange("(c p) n -> p c n", p=P))

            kvK_stage = st_pool.tile([P, DC, nq], qk_dt, name="kvK_stage")
            kvV_stage = st_pool.tile([P, MH, D], FP8_AT, name="kvV_stage")
            co_stage = st_pool.tile([P, NCO, D], BF16, name="co_stage")

            # K first so its AllGather launches as early as possible
            for s0 in range(0, nq, 512):
                for ob in range(DC):
                    ps = ps1.tile([P, 512], F32, name="ps_k", tag="ps1")
                    for c in range(DC):
                        nc.tensor.matmul(
                            ps,
                            lhsT=w_sb["wk"][:, c, ob * P : (ob + 1) * P],
                            rhs=xBT_sb[:, c, s0 : s0 + 512],
                            start=(c == 0),
                            stop=(c == DC - 1),
                        )
                    nc.scalar.activation(
                        out=kvK_stage[:, ob, s0 : s0 + 512],
                        in_=ps,
                        func=AF.Identity,
                        bias=bk_sb[:, ob : ob + 1],
                        scale=1.0,
                    )
            nc.sync.dma_start(out=k_mine, in_=kvK_stage)
            nc.gpsimd.collective_compute(
                "AllGather",
                mybir.AluOpType.bypass,
                replica_groups=KV_GROUPS,
                ins=[k_mine],
                outs=[k_all],
            )
            for h in range(2):
                nc.sync.dma_start(
                    out=kT_sb[:, :, h * nq : (h + 1) * nq],
                    in_=k_all[h].rearrange("p (c m) -> p c m", c=DC),
                )

            # V (own half), stored e5m2 for the fp8 PV matmul
            for m in range(MH):
                ps = ps1.tile([P, 512], F32, name="ps_v", tag="ps1")
                for c in range(DC):
                    nc.tensor.matmul(
                        ps,
                        lhsT=xAT_sb[:, c, m * P : (m + 1) * P],
                        rhs=w_sb["wv"][:, c, :],
                        start=(c == 0),
                        stop=(c == DC - 1),
                    )
                nc.scalar.activation(out=kvV_stage[:, m, :], in_=ps, func=AF.Copy)
            nc.sync.dma_start(out=v_mine, in_=kvV_stage)
            nc.gpsimd.collective_compute(
                "AllGather",
                mybir.AluOpType.bypass,
                replica_groups=KV_GROUPS,
                ins=[v_mine],
                outs=[v_all],
            )
            for h in range(2):
                nc.sync.dma_start(
                    out=v_sb[:, h * MH : (h + 1) * MH, :],
                    in_=v_all[h].rearrange("p (m o) -> p m o", m=MH),
                )

            # co half: the two directions of a pair compute the SAME values
            # (co is direction-independent), so each computes only its
            # host-assigned half of the rows (xcAT/xcBT) and the pair
            # AllGather merges: group rank 0 = d0 = rows 0..half-1.
            for nb in range(NCO):
                ps = ps1.tile([P, 512], F32, name="ps_c", tag="ps1")
                for c in range(DC):
                    nc.tensor.matmul(
                        ps,
                        lhsT=xcAT_sb[:, c, nb * P : (nb + 1) * P],
                        rhs=w_sb["wcA"][:, c, :],
                        start=(c == 0),
                        stop=False,
                    )
                for c in range(DC):
                    nc.tensor.matmul(
                        ps,
                        lhsT=xcBT_sb[:, c, nb * P : (nb + 1) * P],
                        rhs=w_sb["wcB"][:, c, :],
                        start=False,
                        stop=(c == DC - 1),
                    )
                cadd = co_pool.tile([P, D], F32, name="cadd", tag="cadd")
                nc.vector.tensor_add(cadd, ps, bc_bc)
                nc.scalar.activation(out=co_stage[:, nb, :], in_=cadd, func=AF.Relu)
            nc.sync.dma_start(out=co_mine, in_=co_stage)
            nc.gpsimd.collective_compute(
                "AllGather",
                mybir.AluOpType.bypass,
                replica_groups=CO_GROUPS,
                ins=[co_mine],
                outs=[co_all],
            )

            # Q last (local only; overlaps the in-flight gathers)
            for s0 in range(0, nq, 512):
                for ob in range(DC):
                    ps = ps1.tile([P, 512], F32, name="ps_q", tag="ps1")
                    for c in range(DC):
                        nc.tensor.matmul(
                            ps,
                            lhsT=w_sb["wq"][:, c, ob * P : (ob + 1) * P],
                            rhs=xAT_sb[:, c, s0 : s0 + 512],
                            start=(c == 0),
                            stop=(c == DC - 1),
                        )
                    nc.scalar.activation(
                        out=qT_sb[:, ob, s0 : s0 + 512],
                        in_=ps,
                        func=AF.Identity,
                        bias=bq_sb[:, ob : ob + 1],
                        scale=1.0,
                    )

            # land the gathered co halves and add bv (host pre-scales by
            # 1/sqrt(D); attention rows sum to 1 so bv enters via +bv/sqrt(D))
            co_land = st_pool.tile([P, NBQ, D], BF16, name="co_land")
            for g in range(2):
                nc.sync.dma_start(
                    out=co_land[:, g * NCO : (g + 1) * NCO, :],
                    in_=co_all[g].rearrange("p (m o) -> p m o", m=NCO),
                )
            for nb in range(NBQ):
                nc.vector.tensor_add(co_sb[:, nb, :], co_land[:, nb, :], bv_bc)

        # ---------------- phase 2: attention (S^T layout) ----------------
        # Scores are computed transposed (keys on partitions): the exp output
        # is already the [key, query] layout the PV matmul needs as its
        # stationary operand.  Row sums come from a ones-stationary matmul
        # over the bf16 exp tiles; their reciprocals are broadcast across
        # partitions via a DRAM bounce and multiplied in on DVE, giving
        # L1-normalized probabilities in [0,1] that quantize safely to
        # fp8-e5m2 for the DoubleRow PV matmul.
        at_pool = st.enter_context(tc.tile_pool(name="at_pool", bufs=2))
        a8_pool = st.enter_context(tc.tile_pool(name="a8_pool", bufs=2))
        r_pool = st.enter_context(tc.tile_pool(name="r_pool", bufs=2))
        o_pool = st.enter_context(tc.tile_pool(name="o_pool", bufs=3))
        sps_pool = st.enter_context(tc.tile_pool(name="sps", bufs=3, space="PSUM"))
        sum_pool = st.enter_context(tc.tile_pool(name="sump", bufs=2, space="PSUM"))
        pv_pool = st.enter_context(tc.tile_pool(name="pv", bufs=2, space="PSUM"))

        for s0 in range(0, nq, 512):
            at_sb = at_pool.tile([P, MCK, 512], BF16, name="at_sb", tag="at")
            at8 = a8_pool.tile([P, MCK, 512], FP8_AT, name="at8", tag="at8")
            ssum = sum_pool.tile([1, 512], F32, name="ssum", tag="ssum")
            for mb in range(MCK):
                sps = sps_pool.tile([P, 512], F32, name="sps", tag="sps")
                if fp8_scores:
                    for c2 in range(DC // 2):
                        nc.tensor.matmul(
                            sps,
                            lhsT=kT_sb[:, 2 * c2 : 2 * c2 + 2, mb * P : (mb + 1) * P],
                            rhs=qT_sb[:, 2 * c2 : 2 * c2 + 2, s0 : s0 + 512],
                            start=(c2 == 0),
                            stop=(c2 == DC // 2 - 1),
                            perf_mode=DR,
                        )
                else:
                    for c in range(DC):
                        nc.tensor.matmul(
                            sps,
                            lhsT=kT_sb[:, c, mb * P : (mb + 1) * P],
                            rhs=qT_sb[:, c, s0 : s0 + 512],
                            start=(c == 0),
                            stop=(c == DC - 1),
                        )
                nc.scalar.activation(
                    out=at_sb[:, mb, :],
                    in_=sps,
                    func=AF.Exp,
                    bias=shift_sb,
                    scale=1.0,
                )
                # L1 row sums over keys, accumulated across key chunks
                nc.tensor.matmul(
                    ssum,
                    lhsT=ones_col,
                    rhs=at_sb[:, mb, :],
                    start=(mb == 0),
                    stop=(mb == MCK - 1),
                )
            # 1/rowsum, broadcast to all partitions via DRAM
            rs_row = r_pool.tile([1, 512], F32, name="rs_row", tag="rsr")
            nc.vector.reciprocal(out=rs_row, in_=ssum)
            nc.sync.dma_start(out=rs_dram, in_=rs_row)
            rs_bc = r_pool.tile([P, 512], F32, name="rs_bc", tag="rsb")
            nc.sync.dma_start(
                out=rs_bc,
                in_=bass.AP(
                    tensor=rs_dram.tensor,
                    offset=rs_dram.offset,
                    ap=[[0, P]] + list(rs_dram.ap),
                ),
            )
            # normalize -> fp8 probabilities
            for mb in range(MCK):
                nc.vector.tensor_mul(at8[:, mb, :], at_sb[:, mb, :], rs_bc)

            for j in range(4):
                pv = pv_pool.tile([P, D], F32, name="pv", tag="pv")
                for i2 in range(MCK // 2):
                    nc.tensor.matmul(
                        pv,
                        lhsT=at8[:, 2 * i2 : 2 * i2 + 2, j * P : (j + 1) * P],
                        rhs=v_sb[:, 2 * i2 : 2 * i2 + 2, :],
                        start=(i2 == 0),
                        stop=(i2 == MCK // 2 - 1),
                        perf_mode=DR,
                    )
                nb = s0 // P + j
                outt = o_pool.tile([P, D], F32, name="outt", tag="outt")
                nc.vector.scalar_tensor_tensor(
                    out=outt,
                    in0=pv,
                    scalar=INV_SQRT_D,
                    in1=co_sb[:, nb, :],
                    op0=mybir.AluOpType.mult,
                    op1=mybir.AluOpType.add,
                )
                nc.sync.dma_start(out=out[nb * P : (nb + 1) * P, :], in_=outt)


_PROG_CACHE = {}


def _get_program(nq, nkv):
    key = (nq, nkv)
    if key not in _PROG_CACHE:
        _PROG_CACHE[key] = build_program(nq, nkv)
    return _PROG_CACHE[key]


def make_in_maps(x_f, x_s, Wq, bq, Wk, bk, Wv, bv, Wc, bc):
    """Per-core SPMD input dicts + (direction, batch, half) layout.

    x is shipped pre-transposed (feature-major) in bf16.  The co MLP is
    split across direction pairs (its value is direction-independent):
    xcAT/xcBT carry the core's assigned half of its query rows -- first
    half for d=0 cores, second half for d=1 -- and the pair AllGather
    reassembles the full block on both cores in query-row order.
    """
    x_f = np.asarray(x_f, np.float32)
    x_s = np.asarray(x_s, np.float32)
    B, N, _ = x_f.shape
    nq = N // 2
    bf = ml_dtypes.bfloat16
    WqT = np.ascontiguousarray(np.asarray(Wq, np.float32).T).astype(bf)
    WkT = np.ascontiguousarray(np.asarray(Wk, np.float32).T).astype(bf)
    WvT = np.ascontiguousarray(np.asarray(Wv, np.float32).T).astype(bf)
    Wc = np.asarray(Wc, np.float32)
    WcfT = np.ascontiguousarray(Wc[:, :D].T).astype(bf)
    WcsT = np.ascontiguousarray(Wc[:, D:].T).astype(bf)
    bq32, bk32, bv32, bc32 = (
        np.ascontiguousarray(np.asarray(b, np.float32)) for b in (bq, bk, bv, bc)
    )
    # Attention rows sum to 1/sqrt(D) after scaling, so bv enters the output
    # as bv/sqrt(D); the kernel adds its "bv" input to co directly.
    bv32 = np.ascontiguousarray(bv32 / np.sqrt(D, dtype=np.float32))
    half = nq // 2
    in_maps, layout = [], []
    for d in range(2):
        for b in range(B):
            for h in range(2):
                xq = x_f[b] if d == 0 else x_s[b]
                xk = x_s[b] if d == 0 else x_f[b]
                if h == 1:
                    idx = np.r_[nq:N, 0:nq]
                    xq, xk = xq[idx], xk[idx]
                xq, xk = xq[:nq], xk[:nq]  # own query-half rows only
                cosl = slice(0, half) if d == 0 else slice(half, nq)
                in_maps.append(
                    {
                        "xAT": np.ascontiguousarray(xq.T).astype(bf),
                        "xBT": np.ascontiguousarray(xk.T).astype(bf),
                        "xcAT": np.ascontiguousarray(xq[cosl].T).astype(bf),
                        "xcBT": np.ascontiguousarray(xk[cosl].T).astype(bf),
                        "wqT": WqT,
                        "wkT": WkT,
                        "wvT": WvT,
                        "wcAT": WcfT if d == 0 else WcsT,
                        "wcBT": WcsT if d == 0 else WcfT,
                        "bq": bq32,
                        "bk": bk32,
                        "bv": bv32,
                        "bc": bc32,
                    }
                )
                layout.append((d, b, h))
    return in_maps, layout


def kernel(x_f, x_s, Wq, bq, Wk, bk, Wv, bv, Wc, bc):
    x_f = np.asarray(x_f, np.float32)
    B, N, _ = x_f.shape
    nq = N // 2
    nc = _get_program(nq, N)
    in_maps, layout = make_in_maps(x_f, x_s, Wq, bq, Wk, bk, Wv, bv, Wc, bc)

    from concourse.bass_utils import run_bass_kernel_spmd

    res = run_bass_kernel_spmd(nc, in_maps, list(range(len(in_maps))))
    out_f = np.empty((B, N, D), np.float32)
    out_s = np.empty((B, N, D), np.float32)
    for (d, b, h), r in zip(layout, res.results):
        tgt = out_f if d == 0 else out_s
        tgt[b, h * nq : (h + 1) * nq] = r["out"]
    return out_f, out_s


# revision 12
# speedup vs baseline: 4.0559x; 3.5472x over previous
"""Cross-view attention (nn_CrossViewAttention) Trainium2 Bass kernel.

Reference computation (B=2, N=4096, D=512):
    co    = relu(concat([x_f, x_s], -1) @ Wc.T + bc)
    out_f = attend(x_f@Wq.T+bq, x_s@Wk.T+bk, x_f@Wv.T+bv) + co
    out_s = attend(x_s@Wq.T+bq, x_f@Wk.T+bk, x_s@Wv.T+bv) + co
    attend(Q,K,V) = (softmax(Q K^T) / L1 / sqrt(D)) @ V

Sharding: 8 cores = (direction f/s) x (batch 0/1) x (sequence half).
Each core computes 2048 output rows of one direction against the full
4096-row K/V for its (direction, batch), SPMD with per-core input data.
Rows are permuted host-side so every core's own rows come first; the
attention reduction over keys is permutation invariant.  K, V and co
are computed fully locally -- collectives proved to serialize the
whole kernel (~67us each in the cost model) for only ~41us of PE
savings, so there are none.

Design notes:
  - x arrives pre-transposed (feature-major) in bf16 from the host: no
    on-device PE transposes.
  - Q/K are projected in bf16 but stored fp8-e4m3; the N x N score
    matmul runs in fp8 DoubleRow mode (K=256 per instruction).
  - exp runs on ACT into bf16 tiles (scores reach ~71, far outside any
    fp8 range); row sums come from a ones-stationary matmul over those
    tiles; the reciprocals are broadcast across partitions via a DRAM
    bounce and multiplied in on DVE, giving L1-normalized
    probabilities in [0,1] that quantize safely to fp8-e5m2.  The PV
    matmul then runs in fp8 DoubleRow with e5m2 V, and needs no
    post-normalization: the output is one fused DVE op
    out = pv * (1/sqrt(D)) + co.
"""

import sys
from contextlib import ExitStack

for _p in ("/opt/trn_rl_repo", "/root/.axon_site/_ro/trn_rl_repo"):
    if _p not in sys.path:
        sys.path.insert(0, _p)

import ml_dtypes
import numpy as np

import concourse.bacc as bacc
import concourse.bass as bass
import concourse.mybir as mybir
import concourse.tile as tile

P = 128
D = 512
DC = D // P  # contraction chunks of 128
INV_SQRT_D = float(1.0 / np.sqrt(D))
EXP_SHIFT = -40.0

F32 = mybir.dt.float32
BF16 = mybir.dt.bfloat16
FP8_QK = mybir.dt.float8e4
FP8_AT = mybir.dt.float8e5
AF = mybir.ActivationFunctionType
DR = mybir.MatmulPerfMode.DoubleRow


def build_program(nq, nkv, reps=1, fp8_scores=True):
    nc = bacc.Bacc("TRN2", target_bir_lowering=False, debug=False, num_devices=8)

    # feature-major bf16 views, own query-half columns first
    xAT = nc.dram_tensor("xAT", [D, nkv], BF16, kind="ExternalInput").ap()
    xBT = nc.dram_tensor("xBT", [D, nkv], BF16, kind="ExternalInput").ap()
    wqT = nc.dram_tensor("wqT", [D, D], BF16, kind="ExternalInput").ap()
    wkT = nc.dram_tensor("wkT", [D, D], BF16, kind="ExternalInput").ap()
    wvT = nc.dram_tensor("wvT", [D, D], BF16, kind="ExternalInput").ap()
    wcAT = nc.dram_tensor("wcAT", [D, D], BF16, kind="ExternalInput").ap()
    wcBT = nc.dram_tensor("wcBT", [D, D], BF16, kind="ExternalInput").ap()
    bq = nc.dram_tensor("bq", [D], F32, kind="ExternalInput").ap()
    bk = nc.dram_tensor("bk", [D], F32, kind="ExternalInput").ap()
    bv = nc.dram_tensor("bv", [D], F32, kind="ExternalInput").ap()
    bc = nc.dram_tensor("bc", [D], F32, kind="ExternalInput").ap()
    out = nc.dram_tensor("out", [nq, D], F32, kind="ExternalOutput").ap()

    with tile.TileContext(nc) as tc:
        for rep in range(reps):
            _emit_body(
                nc, tc, xAT, xBT, wqT, wkT, wvT, wcAT, wcBT,
                bq, bk, bv, bc, out, nq, nkv, fp8_scores, rep,
            )

    nc.compile()
    return nc


def _emit_body(
    nc, tc, xAT, xBT, wqT, wkT, wvT, wcAT, wcBT,
    bq, bk, bv, bc, out, nq, nkv, fp8_scores, rep,
):
    NBQ = nq // P   # query row blocks (16)
    MCK = nkv // P  # key row chunks (32)
    qk_dt = FP8_QK if fp8_scores else BF16

    rs_dram = nc.dram_tensor(f"rs_dram_{rep}", [512], F32).ap()

    with ExitStack() as st:
        persist = st.enter_context(tc.tile_pool(name="persist", bufs=1))

        w_sb = {}
        for nm, ap_ in (
            ("wq", wqT), ("wk", wkT), ("wv", wvT), ("wcA", wcAT), ("wcB", wcBT),
        ):
            t = persist.tile([P, DC, D], BF16, name=f"w_{nm}")
            nc.sync.dma_start(out=t, in_=ap_.rearrange("(c p) o -> p c o", p=P))
            w_sb[nm] = t

        bq_sb = persist.tile([P, DC], F32, name="bq_sb")
        bk_sb = persist.tile([P, DC], F32, name="bk_sb")
        for ob in range(DC):
            nc.sync.dma_start(
                out=bq_sb[:, ob : ob + 1], in_=bq[ob * P : (ob + 1) * P][:, None]
            )
            nc.sync.dma_start(
                out=bk_sb[:, ob : ob + 1], in_=bk[ob * P : (ob + 1) * P][:, None]
            )

        bv_bc = persist.tile([P, D], F32, name="bv_bc")
        nc.sync.dma_start(
            out=bv_bc,
            in_=bass.AP(tensor=bv.tensor, offset=bv.offset, ap=[[0, P]] + list(bv.ap)),
        )
        bc_bc = persist.tile([P, D], F32, name="bc_bc")
        nc.sync.dma_start(
            out=bc_bc,
            in_=bass.AP(tensor=bc.tensor, offset=bc.offset, ap=[[0, P]] + list(bc.ap)),
        )
        ones_col = persist.tile([P, 1], BF16, name="ones_col")
        nc.vector.memset(ones_col, 1.0)
        shift_sb = persist.tile([P, 1], F32, name="shift_sb")
        nc.vector.memset(shift_sb, EXP_SHIFT)

        qT_sb = persist.tile([P, DC, nq], qk_dt, name="qT_sb")
        kT_sb = persist.tile([P, DC, nkv], qk_dt, name="kT_sb")
        v_sb = persist.tile([P, MCK, D], FP8_AT, name="v_sb")
        co_sb = persist.tile([P, NBQ, D], BF16, name="co_sb")

        # ---------------- phase 1: projections (all local) ----------------
        with ExitStack() as ph1:
            xst_pool = ph1.enter_context(tc.tile_pool(name="xst", bufs=1))
            co_pool = ph1.enter_context(tc.tile_pool(name="cop", bufs=3))
            ps1 = ph1.enter_context(tc.tile_pool(name="ps1", bufs=4, space="PSUM"))

            xAT_sb = xst_pool.tile([P, DC, nkv], BF16, name="xAT_sb")
            nc.sync.dma_start(out=xAT_sb, in_=xAT.rearrange("(c p) n -> p c n", p=P))
            xBT_sb = xst_pool.tile([P, DC, nkv], BF16, name="xBT_sb")
            nc.sync.dma_start(out=xBT_sb, in_=xBT.rearrange("(c p) n -> p c n", p=P))

            # K over all keys, then Q (scores need both; emitted first so
            # phase 2 can start as soon as V/co still stream behind them)
            for s0 in range(0, nkv, 512):
                for ob in range(DC):
                    ps = ps1.tile([P, 512], F32, name="ps_k", tag="ps1")
                    for c in range(DC):
                        nc.tensor.matmul(
                            ps,
                            lhsT=w_sb["wk"][:, c, ob * P : (ob + 1) * P],
                            rhs=xBT_sb[:, c, s0 : s0 + 512],
                            start=(c == 0),
                            stop=(c == DC - 1),
                        )
                    nc.scalar.activation(
                        out=kT_sb[:, ob, s0 : s0 + 512],
                        in_=ps,
                        func=AF.Identity,
                        bias=bk_sb[:, ob : ob + 1],
                        scale=1.0,
                    )
            for s0 in range(0, nq, 512):
                for ob in range(DC):
                    ps = ps1.tile([P, 512], F32, name="ps_q", tag="ps1")
                    for c in range(DC):
                        nc.tensor.matmul(
                            ps,
                            lhsT=w_sb["wq"][:, c, ob * P : (ob + 1) * P],
                            rhs=xAT_sb[:, c, s0 : s0 + 512],
                            start=(c == 0),
                            stop=(c == DC - 1),
                        )
                    nc.scalar.activation(
                        out=qT_sb[:, ob, s0 : s0 + 512],
                        in_=ps,
                        func=AF.Identity,
                        bias=bq_sb[:, ob : ob + 1],
                        scale=1.0,
                    )

            # V over all key rows of the A view, stored e5m2 for fp8 PV;
            # bv is NOT added here -- attention rows sum to 1 after
            # normalization, so bv enters as +bv/sqrt(D) via co instead.
            for m in range(MCK):
                ps = ps1.tile([P, 512], F32, name="ps_v", tag="ps1")
                for c in range(DC):
                    nc.tensor.matmul(
                        ps,
                        lhsT=xAT_sb[:, c, m * P : (m + 1) * P],
                        rhs=w_sb["wv"][:, c, :],
                        start=(c == 0),
                        stop=(c == DC - 1),
                    )
                nc.scalar.activation(out=v_sb[:, m, :], in_=ps, func=AF.Copy)

            # co = relu(xA@WcA.T + xB@WcB.T + bc) + bv/sqrt(D), own rows
            for nb in range(NBQ):
                ps = ps1.tile([P, 512], F32, name="ps_c", tag="ps1")
                for c in range(DC):
                    nc.tensor.matmul(
                        ps,
                        lhsT=xAT_sb[:, c, nb * P : (nb + 1) * P],
                        rhs=w_sb["wcA"][:, c, :],
                        start=(c == 0),
                        stop=False,
                    )
                for c in range(DC):
                    nc.tensor.matmul(
                        ps,
                        lhsT=xBT_sb[:, c, nb * P : (nb + 1) * P],
                        rhs=w_sb["wcB"][:, c, :],
                        start=False,
                        stop=(c == DC - 1),
                    )
                cadd = co_pool.tile([P, D], F32, name="cadd", tag="cadd")
                nc.vector.tensor_add(cadd, ps, bc_bc)
                crl = co_pool.tile([P, D], F32, name="crl", tag="crl")
                nc.scalar.activation(out=crl, in_=cadd, func=AF.Relu)
                nc.vector.tensor_add(co_sb[:, nb, :], crl, bv_bc)

        # ---------------- phase 2: attention (S^T layout) ----------------
        at_pool = st.enter_context(tc.tile_pool(name="at_pool", bufs=2))
        a8_pool = st.enter_context(tc.tile_pool(name="a8_pool", bufs=2))
        r_pool = st.enter_context(tc.tile_pool(name="r_pool", bufs=2))
        o_pool = st.enter_context(tc.tile_pool(name="o_pool", bufs=3))
        sps_pool = st.enter_context(tc.tile_pool(name="sps", bufs=3, space="PSUM"))
        sum_pool = st.enter_context(tc.tile_pool(name="sump", bufs=2, space="PSUM"))
        pv_pool = st.enter_context(tc.tile_pool(name="pv", bufs=2, space="PSUM"))

        for s0 in range(0, nq, 512):
            at_sb = at_pool.tile([P, MCK, 512], BF16, name="at_sb", tag="at")
            at8 = a8_pool.tile([P, MCK, 512], FP8_AT, name="at8", tag="at8")
            ssum = sum_pool.tile([1, 512], F32, name="ssum", tag="ssum")
            for mb in range(MCK):
                sps = sps_pool.tile([P, 512], F32, name="sps", tag="sps")
                if fp8_scores:
                    for c2 in range(DC // 2):
                        nc.tensor.matmul(
                            sps,
                            lhsT=kT_sb[:, 2 * c2 : 2 * c2 + 2, mb * P : (mb + 1) * P],
                            rhs=qT_sb[:, 2 * c2 : 2 * c2 + 2, s0 : s0 + 512],
                            start=(c2 == 0),
                            stop=(c2 == DC // 2 - 1),
                            perf_mode=DR,
                        )
                else:
                    for c in range(DC):
                        nc.tensor.matmul(
                            sps,
                            lhsT=kT_sb[:, c, mb * P : (mb + 1) * P],
                            rhs=qT_sb[:, c, s0 : s0 + 512],
                            start=(c == 0),
                            stop=(c == DC - 1),
                        )
                nc.scalar.activation(
                    out=at_sb[:, mb, :],
                    in_=sps,
                    func=AF.Exp,
                    bias=shift_sb,
                    scale=1.0,
                )
                # L1 row sums over keys, accumulated across key chunks
                nc.tensor.matmul(
                    ssum,
                    lhsT=ones_col,
                    rhs=at_sb[:, mb, :],
                    start=(mb == 0),
                    stop=(mb == MCK - 1),
                )
            # 1/rowsum, broadcast to all partitions via DRAM
            rs_row = r_pool.tile([1, 512], F32, name="rs_row", tag="rsr")
            nc.vector.reciprocal(out=rs_row, in_=ssum)
            nc.sync.dma_start(out=rs_dram, in_=rs_row)
            rs_bc = r_pool.tile([P, 512], F32, name="rs_bc", tag="rsb")
            nc.sync.dma_start(
                out=rs_bc,
                in_=bass.AP(
                    tensor=rs_dram.tensor,
                    offset=rs_dram.offset,
                    ap=[[0, P]] + list(rs_dram.ap),
                ),
            )
            # normalize -> fp8 probabilities
            for mb in range(MCK):
                nc.vector.tensor_mul(at8[:, mb, :], at_sb[:, mb, :], rs_bc)

            for j in range(4):
                pv = pv_pool.tile([P, D], F32, name="pv", tag="pv")
                for i2 in range(MCK // 2):
                    nc.tensor.matmul(
                        pv,
                        lhsT=at8[:, 2 * i2 : 2 * i2 + 2, j * P : (j + 1) * P],
                        rhs=v_sb[:, 2 * i2 : 2 * i2 + 2, :],
                        start=(i2 == 0),
                        stop=(i2 == MCK // 2 - 1),
                        perf_mode=DR,
                    )
                nb = s0 // P + j
                outt = o_pool.tile([P, D], F32, name="outt", tag="outt")
                nc.vector.scalar_tensor_tensor(
                    out=outt,
                    in0=pv,
                    scalar=INV_SQRT_D,
                    in1=co_sb[:, nb, :],
                    op0=mybir.AluOpType.mult,
                    op1=mybir.AluOpType.add,
                )
                nc.sync.dma_start(out=out[nb * P : (nb + 1) * P, :], in_=outt)


_PROG_CACHE = {}


def _get_program(nq, nkv):
    key = (nq, nkv)
    if key not in _PROG_CACHE:
        _PROG_CACHE[key] = build_program(nq, nkv)
    return _PROG_CACHE[key]


def make_in_maps(x_f, x_s, Wq, bq, Wk, bk, Wv, bv, Wc, bc):
    """Per-core SPMD input dicts + (direction, batch, half) layout.

    x is shipped pre-transposed (feature-major) in bf16, full 4096 rows
    per view with the core's own query-half rows first.
    """
    x_f = np.asarray(x_f, np.float32)
    x_s = np.asarray(x_s, np.float32)
    B, N, _ = x_f.shape
    nq = N // 2
    bf = ml_dtypes.bfloat16
    WqT = np.ascontiguousarray(np.asarray(Wq, np.float32).T).astype(bf)
    WkT = np.ascontiguousarray(np.asarray(Wk, np.float32).T).astype(bf)
    WvT = np.ascontiguousarray(np.asarray(Wv, np.float32).T).astype(bf)
    Wc = np.asarray(Wc, np.float32)
    WcfT = np.ascontiguousarray(Wc[:, :D].T).astype(bf)
    WcsT = np.ascontiguousarray(Wc[:, D:].T).astype(bf)
    bq32, bk32, bv32, bc32 = (
        np.ascontiguousarray(np.asarray(b, np.float32)) for b in (bq, bk, bv, bc)
    )
    # Attention rows sum to 1 after L1 normalization and the kernel scales by
    # 1/sqrt(D), so bv enters the output as bv/sqrt(D), added via co.
    bv32 = np.ascontiguousarray(bv32 / np.sqrt(D, dtype=np.float32))
    in_maps, layout = [], []
    for d in range(2):
        for b in range(B):
            for h in range(2):
                xq = x_f[b] if d == 0 else x_s[b]
                xk = x_s[b] if d == 0 else x_f[b]
                if h == 1:
                    idx = np.r_[nq:N, 0:nq]
                    xq, xk = xq[idx], xk[idx]
                in_maps.append(
                    {
                        "xAT": np.ascontiguousarray(xq.T).astype(bf),
                        "xBT": np.ascontiguousarray(xk.T).astype(bf),
                        "wqT": WqT,
                        "wkT": WkT,
                        "wvT": WvT,
                        "wcAT": WcfT if d == 0 else WcsT,
                        "wcBT": WcsT if d == 0 else WcfT,
                        "bq": bq32,
                        "bk": bk32,
                        "bv": bv32,
                        "bc": bc32,
                    }
                )
                layout.append((d, b, h))
    return in_maps, layout


def kernel(x_f, x_s, Wq, bq, Wk, bk, Wv, bv, Wc, bc):
    x_f = np.asarray(x_f, np.float32)
    B, N, _ = x_f.shape
    nq = N // 2
    nc = _get_program(nq, N)
    in_maps, layout = make_in_maps(x_f, x_s, Wq, bq, Wk, bk, Wv, bv, Wc, bc)

    from concourse.bass_utils import run_bass_kernel_spmd

    res = run_bass_kernel_spmd(nc, in_maps, list(range(len(in_maps))))
    out_f = np.empty((B, N, D), np.float32)
    out_s = np.empty((B, N, D), np.float32)
    for (d, b, h), r in zip(layout, res.results):
        tgt = out_f if d == 0 else out_s
        tgt[b, h * nq : (h + 1) * nq] = r["out"]
    return out_f, out_s
